# revision 4
# baseline (speedup 1.0000x reference)
"""GAT-style edge-affinity layer (nn_Decode_Cora) on 8 Trainium2 NeuronCores.

Sharding: each core owns a 512-node slice of the graph. It projects its own
nodes (g = vert @ W), computes attention-numerator/denominator partial sums
over its 512 source nodes j for ALL 4096 destinations i, and a ReduceScatter
(in destination-major layout) hands each core its 512 output rows for the
final divide + ELU.

Math: softmax rows are invariant to per-row scaling, so
    p[i,j] = mask * exp(lrelu(sl_i + sr_j))
           ∝ mask * exp(0.2*sr_j) * exp(relu(0.8*(sl_i+sr_j)))
           = mask * max(exp(0.8*sl_i + sr_j), exp(0.2*sr_j))
which needs only one ACT exp (per-partition bias sr_j) and one fused
scalar_tensor_tensor (max with exp(0.2*sr_j), then multiply by mask).
"""

import sys

for _p in ("/opt/trn_rl_repo",):
    if _p not in sys.path:
        sys.path.append(_p)

import numpy as np
import ml_dtypes

import concourse.bass as bass
import concourse.bacc as bacc
import concourse.mybir as mybir
import concourse.tile as tile
from concourse.masks import make_identity

f32 = mybir.dt.float32
bf16 = mybir.dt.bfloat16

N = 4096          # nodes
F = 1433          # input features
FP = 1536         # padded features (12 * 128)
KT = FP // 128    # 12 contraction tiles
H = 8             # heads
DH = 8            # per-head dim
HD = H * DH       # 64
NC = 8            # cores
NL = N // NC      # 512 nodes per core
NCH = NL // 128   # 4 local j-chunks
NIS = N // 512    # 8 destination column slices
LRELU = 0.2

_STATE = {}


def _build_program():
    nc = bacc.Bacc("TRN2", target_bir_lowering=False, debug=False, num_devices=NC)

    vt = nc.dram_tensor("vt", [FP, NL], f32, kind="ExternalInput")
    wp = nc.dram_tensor("wp", [FP, HD], f32, kind="ExternalInput")
    al8 = nc.dram_tensor("al8", [128, H], f32, kind="ExternalInput")
    ar = nc.dram_tensor("ar", [128, H], f32, kind="ExternalInput")
    mskt = nc.dram_tensor("mskt", [NL, N], bf16, kind="ExternalInput")
    out = nc.dram_tensor("out", [NL, HD], f32, kind="ExternalOutput")

    # internal DRAM for collectives
    sl_loc = nc.dram_tensor("sl_loc", [H, NL], bf16)
    slg = nc.dram_tensor("slg", [NC * H, NL], bf16, addr_space="Shared")
    numt = nc.dram_tensor("numt", [N, 72], f32)
    numt_rs = nc.dram_tensor("numt_rs", [NL, 72], f32)

    with tile.TileContext(nc) as tc:
        with (
            tc.tile_pool(name="const", bufs=1) as cp,
            tc.tile_pool(name="psum", bufs=8, space="PSUM") as pp,
        ):
            # ---- constants / big resident tiles ----
            w_sb = cp.tile([128, KT, HD], f32)
            nc.sync.dma_start(w_sb[:], wp[:].rearrange("(k p) d -> p k d", p=128))
            al_sb = cp.tile([128, H], f32)
            nc.sync.dma_start(al_sb[:], al8[:])
            ar_sb = cp.tile([128, H], f32)
            nc.sync.dma_start(ar_sb[:], ar[:])
            msk_sb = cp.tile([128, NCH, N], bf16)
            nc.sync.dma_start(msk_sb[:], mskt[:].rearrange("(c p) i -> p c i", p=128))
            ident = cp.tile([128, 128], f32)
            make_identity(nc, ident[:])

            gt_sb = cp.tile([128, NL], f32)      # g^T padded to 128 partitions
            nc.vector.memset(gt_sb[64:128, :], 0.0)
            sr_sb = cp.tile([128, NCH * H], f32)   # sr per chunk, col 8c+h
            esr_sb = cp.tile([128, NCH * H], f32)  # exp(0.2 sr)
            gr_sb = cp.tile([128, NCH * 72], bf16)  # lhsT per chunk/head + ones col
            ntb = cp.tile([128, N // 128, 72], f32)  # destination-major num

            # ---- phase 1: projection (vt pool closed early to free SBUF) ----
            with tc.tile_pool(name="vtp", bufs=1) as vtp:
                vts = []
                for k in range(KT):
                    vtt = vtp.tile([128, NL], f32, name=f"vt{k}")
                    nc.sync.dma_start(vtt[:], vt[:].rearrange("(k p) n -> k p n", k=KT)[k])
                    vts.append(vtt)

                gt_ps = pp.tile([128, 512], f32, tag="bank", name="gt_ps")
                for k in range(KT):
                    nc.tensor.matmul(gt_ps[0:HD, :], w_sb[:, k, :], vts[k][:],
                                     start=(k == 0), stop=(k == KT - 1))
                nc.vector.tensor_copy(gt_sb[0:HD, :], gt_ps[0:HD, :])

                # sl8T local: [H, NL] via lhsT=0.8*A_l (padded), rhs=gT
                sl_ps = pp.tile([128, 512], f32, tag="bank", name="sl_ps")
                nc.tensor.matmul(sl_ps[0:H, :], al_sb[:], gt_sb[:], start=True, stop=True)
                sl_sb = cp.tile([H, NL], bf16, name="sl_sb")
                nc.vector.tensor_copy(sl_sb[:], sl_ps[0:H, :])
                nc.sync.dma_start(sl_loc[:], sl_sb[:])

                for c in range(NCH):
                    # g for this chunk: [128 nodes, 64]
                    g_ps = pp.tile([128, 512], f32, tag="bank", name="g_ps")
                    for k in range(KT):
                        nc.tensor.matmul(g_ps[:, 0:HD], vts[k][:, 128 * c:128 * (c + 1)],
                                         w_sb[:, k, :], start=(k == 0), stop=(k == KT - 1))
                    # sr for this chunk
                    sr_ps = pp.tile([128, 512], f32, tag="bank", name="sr_ps")
                    nc.tensor.matmul(sr_ps[:, 0:H], gt_sb[:, 128 * c:128 * (c + 1)],
                                     ar_sb[:], start=True, stop=True)
                    nc.vector.tensor_copy(sr_sb[:, H * c:H * (c + 1)], sr_ps[:, 0:H])
                    nc.scalar.activation(esr_sb[:, H * c:H * (c + 1)], sr_ps[:, 0:H],
                                         mybir.ActivationFunctionType.Exp, scale=0.2)
                    # lhsT tile: [g_h | ones] interleaved, 9 cols per head
                    grc = gr_sb[:, 72 * c:72 * (c + 1)].rearrange("p (h k) -> p h k", k=9)
                    for h in range(H):
                        nc.vector.tensor_copy(grc[:, h, 0:8], g_ps[:, 8 * h:8 * (h + 1)])
                    nc.vector.memset(grc[:, :, 8], 1.0)

            # ---- phase 2: AllGather sl ----
            nc.gpsimd.collective_compute(
                "AllGather", mybir.AluOpType.bypass,
                replica_groups=[list(range(NC))],
                ins=[sl_loc[:].opt()], outs=[slg[:].opt()],
            )

            # ---- phase 3: main attention loop ----
            with (
                tc.tile_pool(name="slbp", bufs=2) as slbp,
                tc.tile_pool(name="tp", bufs=2) as tp,
                tc.tile_pool(name="pmp", bufs=3) as pmp,
                tc.tile_pool(name="nhp", bufs=2) as nhp,
                tc.tile_pool(name="small", bufs=4) as sp,
            ):
                for h in range(H):
                    slb = slbp.tile([128, N], bf16, name="slb")
                    for s in range(NIS):
                        nc.sync.dma_start(
                            slb[:, 512 * s:512 * (s + 1)],
                            slg[H * s + h:H * s + h + 1, :].to_broadcast([128, NL]),
                        )
                    num_ps = [pp.tile([128, 512], f32, tag="bank", name=f"num{s}")
                              for s in range(NIS)]
                    for c in range(NCH):
                        t = tp.tile([128, N], bf16, name="texp")
                        nc.scalar.activation(t[:], slb[:], mybir.ActivationFunctionType.Exp,
                                             bias=sr_sb[:, H * c + h:H * c + h + 1])
                        pm = pmp.tile([128, N], bf16, name="pm")
                        nc.vector.scalar_tensor_tensor(
                            pm[:], t[:], esr_sb[:, H * c + h:H * c + h + 1],
                            msk_sb[:, c, :],
                            mybir.AluOpType.max, mybir.AluOpType.mult,
                        )
                        lhs = gr_sb[:, 72 * c + 9 * h:72 * c + 9 * (h + 1)]
                        for s in range(NIS):
                            nc.tensor.matmul(num_ps[s][0:9, :], lhs,
                                             pm[:, 512 * s:512 * (s + 1)],
                                             start=(c == 0), stop=(c == NCH - 1))
                    numh = nhp.tile([9, N], f32, name="numh")
                    for s in range(NIS):
                        nc.vector.tensor_copy(numh[:, 512 * s:512 * (s + 1)],
                                              num_ps[s][0:9, :])
                    # transpose per 128-destination block into ntb cols 9h:9h+9
                    for b in range(N // 128):
                        tr_ps = pp.tile([128, 512], f32, tag="bank", name="tr_ps")
                        nc.tensor.transpose(tr_ps[:, 0:9], numh[:, 128 * b:128 * (b + 1)],
                                            ident[0:9, 0:9])
                        nc.vector.tensor_copy(ntb[:, b, 9 * h:9 * (h + 1)], tr_ps[:, 0:9])

                # ---- phase 4: write destination-major num to DRAM ----
                for b in range(N // 128):
                    nc.sync.dma_start(numt[128 * b:128 * (b + 1), :], ntb[:, b, :])

                # ---- phase 5: ReduceScatter ----
                nc.gpsimd.collective_compute(
                    "ReduceScatter", mybir.AluOpType.add,
                    replica_groups=[list(range(NC))],
                    ins=[numt[:].opt()], outs=[numt_rs[:].opt()],
                )

                # ---- phase 6: divide + ELU ----
                for b in range(NL // 128):
                    nf = sp.tile([128, 72], f32, name="nf")
                    nc.sync.dma_start(nf[:], numt_rs[128 * b:128 * (b + 1), :])
                    nfr = nf.rearrange("p (h k) -> p h k", k=9)
                    rec = sp.tile([128, H], f32, name="rec")
                    nc.vector.reciprocal(rec[:], nfr[:, :, 8])
                    aout = sp.tile([128, HD], f32, name="aout")
                    for h in range(H):
                        nc.vector.tensor_scalar(aout[:, 8 * h:8 * (h + 1)], nfr[:, h, 0:8],
                                                rec[:, h:h + 1], None, mybir.AluOpType.mult)
                    # elu(x) = relu(x) - 1 + exp(min(x, 0))
                    xm = sp.tile([128, HD], f32, name="xm")
                    nc.vector.tensor_scalar(xm[:], aout[:], 0.0, None, mybir.AluOpType.min)
                    ex = sp.tile([128, HD], f32, name="ex")
                    nc.scalar.activation(ex[:], xm[:], mybir.ActivationFunctionType.Exp)
                    r1 = sp.tile([128, HD], f32, name="r1")
                    nc.vector.tensor_scalar(r1[:], aout[:], 0.0, -1.0,
                                            mybir.AluOpType.max, mybir.AluOpType.add)
                    ot = sp.tile([128, HD], f32, name="ot")
                    nc.vector.tensor_tensor(ot[:], ex[:], r1[:], mybir.AluOpType.add)
                    nc.sync.dma_start(out[128 * b:128 * (b + 1), :], ot[:])

    nc.compile()
    return nc


def _prep_inputs(vert, edge, W, a_l, a_r):
    vert = np.asarray(vert, dtype=np.float32)
    edge = np.asarray(edge)
    W = np.asarray(W, dtype=np.float32)
    a_l = np.asarray(a_l, dtype=np.float32)
    a_r = np.asarray(a_r, dtype=np.float32)

    vtp = np.zeros((FP, N), dtype=np.float32)
    vtp[:F] = vert.T
    wp = np.zeros((FP, HD), dtype=np.float32)
    wp[:F] = W

    al8 = np.zeros((128, H), dtype=np.float32)
    ar8 = np.zeros((128, H), dtype=np.float32)
    for h in range(H):
        al8[8 * h:8 * (h + 1), h] = 0.8 * a_l[h]
        ar8[8 * h:8 * (h + 1), h] = a_r[h]

    maskT = (edge != 0).astype(ml_dtypes.bfloat16)  # [i, j] -> transpose below

    in_maps = []
    for c in range(NC):
        sl = slice(512 * c, 512 * (c + 1))
        in_maps.append({
            "vt": np.ascontiguousarray(vtp[:, sl]),
            "wp": wp,
            "al8": al8,
            "ar": ar8,
            "mskt": np.ascontiguousarray(maskT[:, sl].T),
        })
    return in_maps


def _get_runner():
    """Build (once) and return a callable in_maps -> list of per-core outputs."""
    if "runner" in _STATE:
        return _STATE["runner"]

    nc = _build_program()

    import jax
    from jax.sharding import Mesh, PartitionSpec
    from jax.experimental.shard_map import shard_map
    from concourse import bass2jax
    from concourse.bass2jax import _bass_exec_p, partition_id_tensor

    bass2jax.install_neuronx_cc_hook()

    partition_name = nc.partition_id_tensor.name if nc.partition_id_tensor else None
    in_names, out_names, out_avals, zero_shapes = [], [], [], []
    for alloc in nc.m.functions[0].allocations:
        if not isinstance(alloc, mybir.MemoryLocationSet):
            continue
        name = alloc.memorylocations[0].name
        if alloc.kind == "ExternalInput":
            if name != partition_name:
                in_names.append(name)
        elif alloc.kind == "ExternalOutput":
            shape = tuple(alloc.tensor_shape)
            dtype = mybir.dt.np(alloc.dtype)
            out_names.append(name)
            out_avals.append(jax.core.ShapedArray(shape, dtype))
            zero_shapes.append((shape, dtype))
    n_params = len(in_names)
    n_outs = len(out_avals)
    all_in_names = list(in_names) + list(out_names)
    if partition_name is not None:
        all_in_names.append(partition_name)
    donate = tuple(range(n_params, n_params + n_outs))

    def _body(*args):
        operands = list(args)
        if partition_name is not None:
            operands.append(partition_id_tensor())
        outs = _bass_exec_p.bind(
            *operands,
            out_avals=tuple(out_avals),
            in_names=tuple(all_in_names),
            out_names=tuple(out_names),
            lowering_input_output_aliases=(),
            sim_require_finite=True,
            sim_require_nnan=True,
            nc=nc,
        )
        return tuple(outs)

    devices = jax.devices()[:NC]
    mesh = Mesh(np.asarray(devices), ("core",))
    in_specs = (PartitionSpec("core"),) * (n_params + n_outs)
    out_specs = (PartitionSpec("core"),) * n_outs
    sharded = jax.jit(
        shard_map(_body, mesh=mesh, in_specs=in_specs, out_specs=out_specs,
                  check_rep=False),
        donate_argnums=donate, keep_unused=True,
    )

    def runner(in_maps):
        concat_in = [
            np.concatenate([np.asarray(in_maps[c][nm]) for c in range(NC)], axis=0)
            for nm in in_names
        ]
        concat_zeros = [
            np.zeros((NC * s[0], *s[1:]), dt) for (s, dt) in zero_shapes
        ]
        out_arrs = sharded(*concat_in, *concat_zeros)
        out_arrs = [np.asarray(a) for a in out_arrs]
        return [
            {nm: out_arrs[i].reshape(NC, *out_avals[i].shape)[c]
             for i, nm in enumerate(out_names)}
            for c in range(NC)
        ]

    _STATE["runner"] = runner
    return runner


def kernel(vert, edge, W, a_l, a_r):
    in_maps = _prep_inputs(vert, edge, W, a_l, a_r)
    runner = _get_runner()
    results = runner(in_maps)
    return np.concatenate([results[c]["out"] for c in range(NC)], axis=0)


# revision 5
# speedup vs baseline: 1.0043x; 1.0043x over previous
"""GAT-style edge-affinity layer (nn_Decode_Cora) on 8 Trainium2 NeuronCores.

Sharding: each core owns a 512-node slice of the graph. It projects its own
nodes (g = vert @ W), computes attention-numerator/denominator partial sums
over its 512 source nodes j for ALL 4096 destinations i, and a ReduceScatter
(in destination-major layout) hands each core its 512 output rows for the
final divide + ELU.

Math: softmax rows are invariant to per-row scaling, so
    p[i,j] = mask * exp(lrelu(sl_i + sr_j))
           ∝ mask * exp(0.2*sr_j) * exp(relu(0.8*(sl_i+sr_j)))
           = mask * max(exp(0.8*sl_i + sr_j), exp(0.2*sr_j))
which needs only one ACT exp (per-partition bias sr_j) and one fused
scalar_tensor_tensor (max with exp(0.2*sr_j), then multiply by mask).
"""

import sys

for _p in ("/opt/trn_rl_repo",):
    if _p not in sys.path:
        sys.path.append(_p)

import numpy as np
import ml_dtypes

import concourse.bass as bass
import concourse.bacc as bacc
import concourse.mybir as mybir
import concourse.tile as tile
from concourse.masks import make_identity

f32 = mybir.dt.float32
f16 = mybir.dt.float16

N = 4096          # nodes
F = 1433          # input features
FP = 1536         # padded features (12 * 128)
KT = FP // 128    # 12 contraction tiles
H = 8             # heads
DH = 8            # per-head dim
HD = H * DH       # 64
NC = 8            # cores
NL = N // NC      # 512 nodes per core
NCH = NL // 128   # 4 local j-chunks
NIS = N // 512    # 8 destination column slices
LRELU = 0.2

_STATE = {}


def _build_program():
    nc = bacc.Bacc("TRN2", target_bir_lowering=False, debug=False, num_devices=NC)

    vt = nc.dram_tensor("vt", [FP, NL], f32, kind="ExternalInput")
    wp = nc.dram_tensor("wp", [FP, HD], f32, kind="ExternalInput")
    al8 = nc.dram_tensor("al8", [128, H], f32, kind="ExternalInput")
    ar = nc.dram_tensor("ar", [128, H], f32, kind="ExternalInput")
    mskt = nc.dram_tensor("mskt", [NL, N], f16, kind="ExternalInput")
    out = nc.dram_tensor("out", [NL, HD], f32, kind="ExternalOutput")

    # internal DRAM for collectives
    sl_loc = nc.dram_tensor("sl_loc", [H, NL], f16)
    slg = nc.dram_tensor("slg", [NC * H, NL], f16, addr_space="Shared")
    numt = nc.dram_tensor("numt", [N, 72], f32)
    numt_rs = nc.dram_tensor("numt_rs", [NL, 72], f32)

    with tile.TileContext(nc) as tc:
        with (
            tc.tile_pool(name="const", bufs=1) as cp,
            tc.tile_pool(name="psum", bufs=8, space="PSUM") as pp,
        ):
            # ---- constants / big resident tiles ----
            w_sb = cp.tile([128, KT, HD], f32)
            nc.sync.dma_start(w_sb[:], wp[:].rearrange("(k p) d -> p k d", p=128))
            al_sb = cp.tile([128, H], f32)
            nc.sync.dma_start(al_sb[:], al8[:])
            ar_sb = cp.tile([128, H], f32)
            nc.sync.dma_start(ar_sb[:], ar[:])
            msk_sb = cp.tile([128, NCH, N], f16)
            nc.sync.dma_start(msk_sb[:], mskt[:].rearrange("(c p) i -> p c i", p=128))
            ident = cp.tile([128, 128], f32)
            make_identity(nc, ident[:])

            gt_sb = cp.tile([128, NL], f32)      # g^T padded to 128 partitions
            nc.vector.memset(gt_sb[64:128, :], 0.0)
            sr_sb = cp.tile([128, NCH * H], f32)   # sr per chunk, col 8c+h
            esr_sb = cp.tile([128, NCH * H], f32)  # exp(0.2 sr)
            gr_sb = cp.tile([128, NCH * 72], f16)  # lhsT per chunk/head + ones col
            ntb = cp.tile([128, N // 128, 72], f32)  # destination-major num

            # ---- phase 1: projection (vt pool closed early to free SBUF) ----
            with tc.tile_pool(name="vtp", bufs=1) as vtp:
                vts = []
                for k in range(KT):
                    vtt = vtp.tile([128, NL], f32, name=f"vt{k}")
                    nc.sync.dma_start(vtt[:], vt[:].rearrange("(k p) n -> k p n", k=KT)[k])
                    vts.append(vtt)

                gt_ps = pp.tile([128, 512], f32, tag="bank", name="gt_ps")
                for k in range(KT):
                    nc.tensor.matmul(gt_ps[0:HD, :], w_sb[:, k, :], vts[k][:],
                                     start=(k == 0), stop=(k == KT - 1))
                nc.vector.tensor_copy(gt_sb[0:HD, :], gt_ps[0:HD, :])

                # sl8T local: [H, NL] via lhsT=0.8*A_l (padded), rhs=gT
                sl_ps = pp.tile([128, 512], f32, tag="bank", name="sl_ps")
                nc.tensor.matmul(sl_ps[0:H, :], al_sb[:], gt_sb[:], start=True, stop=True)
                sl_sb = cp.tile([H, NL], f16, name="sl_sb")
                nc.vector.tensor_copy(sl_sb[:], sl_ps[0:H, :])
                nc.sync.dma_start(sl_loc[:], sl_sb[:])

                for c in range(NCH):
                    # g for this chunk: [128 nodes, 64]
                    g_ps = pp.tile([128, 512], f32, tag="bank", name="g_ps")
                    for k in range(KT):
                        nc.tensor.matmul(g_ps[:, 0:HD], vts[k][:, 128 * c:128 * (c + 1)],
                                         w_sb[:, k, :], start=(k == 0), stop=(k == KT - 1))
                    # sr for this chunk
                    sr_ps = pp.tile([128, 512], f32, tag="bank", name="sr_ps")
                    nc.tensor.matmul(sr_ps[:, 0:H], gt_sb[:, 128 * c:128 * (c + 1)],
                                     ar_sb[:], start=True, stop=True)
                    nc.vector.tensor_copy(sr_sb[:, H * c:H * (c + 1)], sr_ps[:, 0:H])
                    nc.scalar.activation(esr_sb[:, H * c:H * (c + 1)], sr_ps[:, 0:H],
                                         mybir.ActivationFunctionType.Exp, scale=0.2)
                    # lhsT tile: [g_h | ones] interleaved, 9 cols per head
                    grc = gr_sb[:, 72 * c:72 * (c + 1)].rearrange("p (h k) -> p h k", k=9)
                    for h in range(H):
                        nc.vector.tensor_copy(grc[:, h, 0:8], g_ps[:, 8 * h:8 * (h + 1)])
                    nc.vector.memset(grc[:, :, 8], 1.0)

            # ---- phase 2: AllGather sl ----
            nc.gpsimd.collective_compute(
                "AllGather", mybir.AluOpType.bypass,
                replica_groups=[list(range(NC))],
                ins=[sl_loc[:].opt()], outs=[slg[:].opt()],
            )

            # ---- phase 3: main attention loop ----
            with (
                tc.tile_pool(name="slbp", bufs=2) as slbp,
                tc.tile_pool(name="tp", bufs=2) as tp,
                tc.tile_pool(name="pmp", bufs=3) as pmp,
                tc.tile_pool(name="nhp", bufs=2) as nhp,
                tc.tile_pool(name="small", bufs=4) as sp,
            ):
                for h in range(H):
                    slb = slbp.tile([128, N], f16, name="slb")
                    for s in range(NIS):
                        nc.sync.dma_start(
                            slb[:, 512 * s:512 * (s + 1)],
                            slg[H * s + h:H * s + h + 1, :].to_broadcast([128, NL]),
                        )
                    num_ps = [pp.tile([128, 512], f32, tag="bank", name=f"num{s}")
                              for s in range(NIS)]
                    for c in range(NCH):
                        t = tp.tile([128, N], f16, name="texp")
                        nc.scalar.activation(t[:], slb[:], mybir.ActivationFunctionType.Exp,
                                             bias=sr_sb[:, H * c + h:H * c + h + 1])
                        pm = pmp.tile([128, N], f16, name="pm")
                        nc.vector.scalar_tensor_tensor(
                            pm[:], t[:], esr_sb[:, H * c + h:H * c + h + 1],
                            msk_sb[:, c, :],
                            mybir.AluOpType.max, mybir.AluOpType.mult,
                        )
                        lhs = gr_sb[:, 72 * c + 9 * h:72 * c + 9 * (h + 1)]
                        for s in range(NIS):
                            nc.tensor.matmul(num_ps[s][0:9, :], lhs,
                                             pm[:, 512 * s:512 * (s + 1)],
                                             start=(c == 0), stop=(c == NCH - 1))
                    numh = nhp.tile([9, N], f32, name="numh")
                    for s in range(NIS):
                        nc.vector.tensor_copy(numh[:, 512 * s:512 * (s + 1)],
                                              num_ps[s][0:9, :])
                    # transpose per 128-destination block into ntb cols 9h:9h+9
                    for b in range(N // 128):
                        tr_ps = pp.tile([128, 512], f32, tag="bank", name="tr_ps")
                        nc.tensor.transpose(tr_ps[:, 0:9], numh[:, 128 * b:128 * (b + 1)],
                                            ident[0:9, 0:9])
                        nc.vector.tensor_copy(ntb[:, b, 9 * h:9 * (h + 1)], tr_ps[:, 0:9])

                # ---- phase 4: write destination-major num to DRAM ----
                for b in range(N // 128):
                    nc.sync.dma_start(numt[128 * b:128 * (b + 1), :], ntb[:, b, :])

                # ---- phase 5: ReduceScatter ----
                nc.gpsimd.collective_compute(
                    "ReduceScatter", mybir.AluOpType.add,
                    replica_groups=[list(range(NC))],
                    ins=[numt[:].opt()], outs=[numt_rs[:].opt()],
                )

                # ---- phase 6: divide + ELU ----
                for b in range(NL // 128):
                    nf = sp.tile([128, 72], f32, name="nf")
                    nc.sync.dma_start(nf[:], numt_rs[128 * b:128 * (b + 1), :])
                    nfr = nf.rearrange("p (h k) -> p h k", k=9)
                    rec = sp.tile([128, H], f32, name="rec")
                    nc.vector.reciprocal(rec[:], nfr[:, :, 8])
                    aout = sp.tile([128, HD], f32, name="aout")
                    for h in range(H):
                        nc.vector.tensor_scalar(aout[:, 8 * h:8 * (h + 1)], nfr[:, h, 0:8],
                                                rec[:, h:h + 1], None, mybir.AluOpType.mult)
                    # elu(x) = relu(x) - 1 + exp(min(x, 0))
                    xm = sp.tile([128, HD], f32, name="xm")
                    nc.vector.tensor_scalar(xm[:], aout[:], 0.0, None, mybir.AluOpType.min)
                    ex = sp.tile([128, HD], f32, name="ex")
                    nc.scalar.activation(ex[:], xm[:], mybir.ActivationFunctionType.Exp)
                    r1 = sp.tile([128, HD], f32, name="r1")
                    nc.vector.tensor_scalar(r1[:], aout[:], 0.0, -1.0,
                                            mybir.AluOpType.max, mybir.AluOpType.add)
                    ot = sp.tile([128, HD], f32, name="ot")
                    nc.vector.tensor_tensor(ot[:], ex[:], r1[:], mybir.AluOpType.add)
                    nc.sync.dma_start(out[128 * b:128 * (b + 1), :], ot[:])

    nc.compile()
    return nc


def _prep_inputs(vert, edge, W, a_l, a_r):
    vert = np.asarray(vert, dtype=np.float32)
    edge = np.asarray(edge)
    W = np.asarray(W, dtype=np.float32)
    a_l = np.asarray(a_l, dtype=np.float32)
    a_r = np.asarray(a_r, dtype=np.float32)

    vtp = np.zeros((FP, N), dtype=np.float32)
    vtp[:F] = vert.T
    wp = np.zeros((FP, HD), dtype=np.float32)
    wp[:F] = W

    al8 = np.zeros((128, H), dtype=np.float32)
    ar8 = np.zeros((128, H), dtype=np.float32)
    for h in range(H):
        al8[8 * h:8 * (h + 1), h] = 0.8 * a_l[h]
        ar8[8 * h:8 * (h + 1), h] = a_r[h]

    maskT = (edge != 0).astype(np.float16)  # [i, j] -> transpose below

    in_maps = []
    for c in range(NC):
        sl = slice(512 * c, 512 * (c + 1))
        in_maps.append({
            "vt": np.ascontiguousarray(vtp[:, sl]),
            "wp": wp,
            "al8": al8,
            "ar": ar8,
            "mskt": np.ascontiguousarray(maskT[:, sl].T),
        })
    return in_maps


def _get_runner():
    """Build (once) and return a callable in_maps -> list of per-core outputs."""
    if "runner" in _STATE:
        return _STATE["runner"]

    nc = _build_program()

    import jax
    from jax.sharding import Mesh, PartitionSpec
    from jax.experimental.shard_map import shard_map
    from concourse import bass2jax
    from concourse.bass2jax import _bass_exec_p, partition_id_tensor

    bass2jax.install_neuronx_cc_hook()

    partition_name = nc.partition_id_tensor.name if nc.partition_id_tensor else None
    in_names, out_names, out_avals, zero_shapes = [], [], [], []
    for alloc in nc.m.functions[0].allocations:
        if not isinstance(alloc, mybir.MemoryLocationSet):
            continue
        name = alloc.memorylocations[0].name
        if alloc.kind == "ExternalInput":
            if name != partition_name:
                in_names.append(name)
        elif alloc.kind == "ExternalOutput":
            shape = tuple(alloc.tensor_shape)
            dtype = mybir.dt.np(alloc.dtype)
            out_names.append(name)
            out_avals.append(jax.core.ShapedArray(shape, dtype))
            zero_shapes.append((shape, dtype))
    n_params = len(in_names)
    n_outs = len(out_avals)
    all_in_names = list(in_names) + list(out_names)
    if partition_name is not None:
        all_in_names.append(partition_name)
    donate = tuple(range(n_params, n_params + n_outs))

    def _body(*args):
        operands = list(args)
        if partition_name is not None:
            operands.append(partition_id_tensor())
        outs = _bass_exec_p.bind(
            *operands,
            out_avals=tuple(out_avals),
            in_names=tuple(all_in_names),
            out_names=tuple(out_names),
            lowering_input_output_aliases=(),
            sim_require_finite=True,
            sim_require_nnan=True,
            nc=nc,
        )
        return tuple(outs)

    devices = jax.devices()[:NC]
    mesh = Mesh(np.asarray(devices), ("core",))
    in_specs = (PartitionSpec("core"),) * (n_params + n_outs)
    out_specs = (PartitionSpec("core"),) * n_outs
    sharded = jax.jit(
        shard_map(_body, mesh=mesh, in_specs=in_specs, out_specs=out_specs,
                  check_rep=False),
        donate_argnums=donate, keep_unused=True,
    )

    def runner(in_maps):
        concat_in = [
            np.concatenate([np.asarray(in_maps[c][nm]) for c in range(NC)], axis=0)
            for nm in in_names
        ]
        concat_zeros = [
            np.zeros((NC * s[0], *s[1:]), dt) for (s, dt) in zero_shapes
        ]
        out_arrs = sharded(*concat_in, *concat_zeros)
        out_arrs = [np.asarray(a) for a in out_arrs]
        return [
            {nm: out_arrs[i].reshape(NC, *out_avals[i].shape)[c]
             for i, nm in enumerate(out_names)}
            for c in range(NC)
        ]

    _STATE["runner"] = runner
    return runner


def kernel(vert, edge, W, a_l, a_r):
    in_maps = _prep_inputs(vert, edge, W, a_l, a_r)
    runner = _get_runner()
    results = runner(in_maps)
    return np.concatenate([results[c]["out"] for c in range(NC)], axis=0)


# revision 6
# speedup vs baseline: 8.6588x; 8.6215x over previous
"""GAT-style edge-affinity layer (nn_Decode_Cora) on 8 Trainium2 NeuronCores.

Sharding: each core owns a 512-node slice of the graph. It projects its own
nodes (g = vert @ W), computes attention-numerator/denominator partial sums
over its 512 source nodes j for ALL 4096 destinations i, and a ReduceScatter
(in destination-major layout) hands each core its 512 output rows for the
final divide + ELU.

Math: softmax rows are invariant to per-row scaling, so
    p[i,j] = mask * exp(lrelu(sl_i + sr_j))
           ∝ mask * exp(0.2*sr_j) * exp(relu(0.8*(sl_i+sr_j)))
           = mask * max(exp(0.8*sl_i + sr_j), exp(0.2*sr_j))
which needs only one ACT exp (per-partition bias sr_j) and one fused
scalar_tensor_tensor (max with exp(0.2*sr_j), then multiply by mask).
"""

import sys

for _p in ("/opt/trn_rl_repo",):
    if _p not in sys.path:
        sys.path.append(_p)

import numpy as np
import ml_dtypes

import concourse.bass as bass
import concourse.bacc as bacc
import concourse.mybir as mybir
import concourse.tile as tile
from concourse.masks import make_identity

f32 = mybir.dt.float32
f16 = mybir.dt.float16

N = 4096          # nodes
F = 1433          # input features
FP = 1536         # padded features (12 * 128)
KT = FP // 128    # 12 contraction tiles
H = 8             # heads
DH = 8            # per-head dim
HD = H * DH       # 64
NC = 8            # cores
NL = N // NC      # 512 nodes per core
NCH = NL // 128   # 4 local j-chunks
NIS = N // 512    # 8 destination column slices
LRELU = 0.2

_STATE = {}


def _build_program():
    nc = bacc.Bacc("TRN2", target_bir_lowering=False, debug=False, num_devices=NC)

    vt = nc.dram_tensor("vt", [FP, NL], f32, kind="ExternalInput")
    wp = nc.dram_tensor("wp", [FP, HD], f32, kind="ExternalInput")
    al8 = nc.dram_tensor("al8", [128, H], f32, kind="ExternalInput")
    ar = nc.dram_tensor("ar", [128, H], f32, kind="ExternalInput")
    mskt = nc.dram_tensor("mskt", [NL, N], f16, kind="ExternalInput")
    out = nc.dram_tensor("out", [NL, HD], f32, kind="ExternalOutput")

    # internal DRAM for collectives
    sl_loc = nc.dram_tensor("sl_loc", [H, NL], f16)
    slg = nc.dram_tensor("slg", [NC * H, NL], f16, addr_space="Shared")
    numt = nc.dram_tensor("numt", [N, 72], f32)
    numt_rs = nc.dram_tensor("numt_rs", [NL, 72], f32)

    with tile.TileContext(nc) as tc:
        with (
            tc.tile_pool(name="const", bufs=1) as cp,
            tc.tile_pool(name="psum", bufs=8, space="PSUM") as pp,
        ):
            # ---- constants / big resident tiles ----
            w_sb = cp.tile([128, KT, HD], f32)
            nc.sync.dma_start(w_sb[:], wp[:].rearrange("(k p) d -> p k d", p=128))
            al_sb = cp.tile([128, H], f32)
            nc.sync.dma_start(al_sb[:], al8[:])
            ar_sb = cp.tile([128, H], f32)
            nc.sync.dma_start(ar_sb[:], ar[:])
            msk_sb = cp.tile([128, NCH, N], f16)
            nc.sync.dma_start(msk_sb[:], mskt[:].rearrange("(c p) i -> p c i", p=128))
            ident = cp.tile([128, 128], f32)
            make_identity(nc, ident[:])

            gt_sb = cp.tile([128, NL], f32)      # g^T padded to 128 partitions
            nc.vector.memset(gt_sb[64:128, :], 0.0)
            sr_sb = cp.tile([128, NCH * H], f32)   # sr per chunk, col 8c+h
            esr_sb = cp.tile([128, NCH * H], f32)  # exp(0.2 sr)
            gr_sb = cp.tile([128, NCH * 72], f16)  # lhsT per chunk/head + ones col
            ntb = cp.tile([128, N // 128, 72], f32)  # destination-major num

            # ---- phase 1: projection (vt pool closed early to free SBUF) ----
            with tc.tile_pool(name="vtp", bufs=1) as vtp:
                vts = []
                for k in range(KT):
                    vtt = vtp.tile([128, NL], f32, name=f"vt{k}")
                    nc.sync.dma_start(vtt[:], vt[:].rearrange("(k p) n -> k p n", k=KT)[k])
                    vts.append(vtt)

                gt_ps = pp.tile([128, 512], f32, tag="bank", name="gt_ps")
                for k in range(KT):
                    nc.tensor.matmul(gt_ps[0:HD, :], w_sb[:, k, :], vts[k][:],
                                     start=(k == 0), stop=(k == KT - 1))
                nc.vector.tensor_copy(gt_sb[0:HD, :], gt_ps[0:HD, :])

                # sl8T local: [H, NL] via lhsT=0.8*A_l (padded), rhs=gT
                sl_ps = pp.tile([128, 512], f32, tag="bank", name="sl_ps")
                nc.tensor.matmul(sl_ps[0:H, :], al_sb[:], gt_sb[:], start=True, stop=True)
                sl_sb = cp.tile([H, NL], f16, name="sl_sb")
                nc.vector.tensor_copy(sl_sb[:], sl_ps[0:H, :])
                nc.sync.dma_start(sl_loc[:], sl_sb[:])

                for c in range(NCH):
                    # g for this chunk: [128 nodes, 64]
                    g_ps = pp.tile([128, 512], f32, tag="bank", name="g_ps")
                    for k in range(KT):
                        nc.tensor.matmul(g_ps[:, 0:HD], vts[k][:, 128 * c:128 * (c + 1)],
                                         w_sb[:, k, :], start=(k == 0), stop=(k == KT - 1))
                    # sr for this chunk
                    sr_ps = pp.tile([128, 512], f32, tag="bank", name="sr_ps")
                    nc.tensor.matmul(sr_ps[:, 0:H], gt_sb[:, 128 * c:128 * (c + 1)],
                                     ar_sb[:], start=True, stop=True)
                    nc.vector.tensor_copy(sr_sb[:, H * c:H * (c + 1)], sr_ps[:, 0:H])
                    nc.scalar.activation(esr_sb[:, H * c:H * (c + 1)], sr_ps[:, 0:H],
                                         mybir.ActivationFunctionType.Exp, scale=0.2)
                    # lhsT tile: [g_h | ones] interleaved, 9 cols per head
                    grc = gr_sb[:, 72 * c:72 * (c + 1)].rearrange("p (h k) -> p h k", k=9)
                    for h in range(H):
                        nc.vector.tensor_copy(grc[:, h, 0:8], g_ps[:, 8 * h:8 * (h + 1)])
                    nc.vector.memset(grc[:, :, 8], 1.0)

            # ---- phase 2: AllGather sl ----
            nc.gpsimd.collective_compute(
                "AllGather", mybir.AluOpType.bypass,
                replica_groups=[list(range(NC))],
                ins=[sl_loc[:].opt()], outs=[slg[:].opt()],
            )

            # ---- phase 3: main attention loop ----
            with (
                tc.tile_pool(name="slbp", bufs=2) as slbp,
                tc.tile_pool(name="tp", bufs=2) as tp,
                tc.tile_pool(name="pmp", bufs=3) as pmp,
                tc.tile_pool(name="nhp", bufs=2) as nhp,
                tc.tile_pool(name="small", bufs=4) as sp,
            ):
                for h in range(H):
                    slb = slbp.tile([128, N], f16, name="slb")
                    for s in range(NIS):
                        nc.sync.dma_start(
                            slb[:, 512 * s:512 * (s + 1)],
                            slg[H * s + h:H * s + h + 1, :].to_broadcast([128, NL]),
                        )
                    num_ps = [pp.tile([128, 512], f32, tag="bank", name=f"num{s}")
                              for s in range(NIS)]
                    for c in range(NCH):
                        t = tp.tile([128, N], f16, name="texp")
                        nc.scalar.activation(t[:], slb[:], mybir.ActivationFunctionType.Exp,
                                             bias=sr_sb[:, H * c + h:H * c + h + 1])
                        pm = pmp.tile([128, N], f16, name="pm")
                        nc.vector.scalar_tensor_tensor(
                            pm[:], t[:], esr_sb[:, H * c + h:H * c + h + 1],
                            msk_sb[:, c, :],
                            mybir.AluOpType.max, mybir.AluOpType.mult,
                        )
                        lhs = gr_sb[:, 72 * c + 9 * h:72 * c + 9 * (h + 1)]
                        for s in range(NIS):
                            nc.tensor.matmul(num_ps[s][0:9, :], lhs,
                                             pm[:, 512 * s:512 * (s + 1)],
                                             start=(c == 0), stop=(c == NCH - 1))
                    numh = nhp.tile([9, N], f32, name="numh")
                    for s in range(NIS):
                        nc.vector.tensor_copy(numh[:, 512 * s:512 * (s + 1)],
                                              num_ps[s][0:9, :])
                    # transpose per 128-destination block into ntb cols 9h:9h+9
                    for b in range(N // 128):
                        tr_ps = pp.tile([128, 512], f32, tag="bank", name="tr_ps")
                        nc.tensor.transpose(tr_ps[:, 0:9], numh[:, 128 * b:128 * (b + 1)],
                                            ident[0:9, 0:9])
                        nc.vector.tensor_copy(ntb[:, b, 9 * h:9 * (h + 1)], tr_ps[:, 0:9])

                # ---- phase 4: write destination-major num to DRAM ----
                for b in range(N // 128):
                    nc.sync.dma_start(numt[128 * b:128 * (b + 1), :], ntb[:, b, :])

                # ---- phase 5: ReduceScatter ----
                nc.gpsimd.collective_compute(
                    "ReduceScatter", mybir.AluOpType.add,
                    replica_groups=[list(range(NC))],
                    ins=[numt[:].opt()], outs=[numt_rs[:].opt()],
                )

                # ---- phase 6: divide + ELU ----
                for b in range(NL // 128):
                    nf = sp.tile([128, 72], f32, name="nf")
                    nc.sync.dma_start(nf[:], numt_rs[128 * b:128 * (b + 1), :])
                    nfr = nf.rearrange("p (h k) -> p h k", k=9)
                    rec = sp.tile([128, H], f32, name="rec")
                    nc.vector.reciprocal(rec[:], nfr[:, :, 8])
                    aout = sp.tile([128, HD], f32, name="aout")
                    for h in range(H):
                        nc.vector.tensor_scalar(aout[:, 8 * h:8 * (h + 1)], nfr[:, h, 0:8],
                                                rec[:, h:h + 1], None, mybir.AluOpType.mult)
                    # elu(x) = relu(x) - 1 + exp(min(x, 0))
                    xm = sp.tile([128, HD], f32, name="xm")
                    nc.vector.tensor_scalar(xm[:], aout[:], 0.0, None, mybir.AluOpType.min)
                    ex = sp.tile([128, HD], f32, name="ex")
                    nc.scalar.activation(ex[:], xm[:], mybir.ActivationFunctionType.Exp)
                    r1 = sp.tile([128, HD], f32, name="r1")
                    nc.vector.tensor_scalar(r1[:], aout[:], 0.0, -1.0,
                                            mybir.AluOpType.max, mybir.AluOpType.add)
                    ot = sp.tile([128, HD], f32, name="ot")
                    nc.vector.tensor_tensor(ot[:], ex[:], r1[:], mybir.AluOpType.add)
                    nc.sync.dma_start(out[128 * b:128 * (b + 1), :], ot[:])

    nc.compile()
    return nc


def _prep_inputs(vert, edge, W, a_l, a_r):
    vert = np.asarray(vert, dtype=np.float32)
    edge = np.asarray(edge)
    W = np.asarray(W, dtype=np.float32)
    a_l = np.asarray(a_l, dtype=np.float32)
    a_r = np.asarray(a_r, dtype=np.float32)

    vtp = np.zeros((FP, N), dtype=np.float32)
    vtp[:F] = vert.T
    wp = np.zeros((FP, HD), dtype=np.float32)
    wp[:F] = W

    al8 = np.zeros((128, H), dtype=np.float32)
    ar8 = np.zeros((128, H), dtype=np.float32)
    for h in range(H):
        al8[8 * h:8 * (h + 1), h] = 0.8 * a_l[h]
        ar8[8 * h:8 * (h + 1), h] = a_r[h]

    maskT = (edge != 0).astype(np.float16)  # [i, j] -> transpose below

    in_maps = []
    for c in range(NC):
        sl = slice(512 * c, 512 * (c + 1))
        in_maps.append({
            "vt": np.ascontiguousarray(vtp[:, sl]),
            "wp": wp,
            "al8": al8,
            "ar": ar8,
            "mskt": np.ascontiguousarray(maskT[:, sl].T),
        })
    return in_maps


def _get_runner():
    """Build (once) and return a callable in_maps -> list of per-core outputs."""
    if "runner" in _STATE:
        return _STATE["runner"]

    nc = _build_program()

    import jax
    from jax.sharding import Mesh, PartitionSpec
    from jax.experimental.shard_map import shard_map
    from concourse import bass2jax
    from concourse.bass2jax import _bass_exec_p, partition_id_tensor

    bass2jax.install_neuronx_cc_hook()

    partition_name = nc.partition_id_tensor.name if nc.partition_id_tensor else None
    in_names, out_names, out_avals, zero_shapes = [], [], [], []
    for alloc in nc.m.functions[0].allocations:
        if not isinstance(alloc, mybir.MemoryLocationSet):
            continue
        name = alloc.memorylocations[0].name
        if alloc.kind == "ExternalInput":
            if name != partition_name:
                in_names.append(name)
        elif alloc.kind == "ExternalOutput":
            shape = tuple(alloc.tensor_shape)
            dtype = mybir.dt.np(alloc.dtype)
            out_names.append(name)
            out_avals.append(jax.core.ShapedArray(shape, dtype))
            zero_shapes.append((shape, dtype))
    n_params = len(in_names)
    n_outs = len(out_avals)
    all_in_names = list(in_names) + list(out_names)
    if partition_name is not None:
        all_in_names.append(partition_name)
    donate = tuple(range(n_params, n_params + n_outs))

    def _body(*args):
        operands = list(args)
        if partition_name is not None:
            operands.append(partition_id_tensor())
        outs = _bass_exec_p.bind(
            *operands,
            out_avals=tuple(out_avals),
            in_names=tuple(all_in_names),
            out_names=tuple(out_names),
            lowering_input_output_aliases=(),
            sim_require_finite=True,
            sim_require_nnan=True,
            nc=nc,
        )
        return tuple(outs)

    devices = jax.devices()[:NC]
    mesh = Mesh(np.asarray(devices), ("core",))
    in_specs = (PartitionSpec("core"),) * (n_params + n_outs)
    out_specs = (PartitionSpec("core"),) * n_outs
    sharded = jax.jit(
        shard_map(_body, mesh=mesh, in_specs=in_specs, out_specs=out_specs,
                  check_rep=False),
        donate_argnums=donate, keep_unused=True,
    )

    def runner(in_maps):
        concat_in = [
            np.concatenate([np.asarray(in_maps[c][nm]) for c in range(NC)], axis=0)
            for nm in in_names
        ]
        concat_zeros = [
            np.zeros((NC * s[0], *s[1:]), dt) for (s, dt) in zero_shapes
        ]
        out_arrs = sharded(*concat_in, *concat_zeros)
        out_arrs = [np.asarray(a) for a in out_arrs]
        return [
            {nm: out_arrs[i].reshape(NC, *out_avals[i].shape)[c]
             for i, nm in enumerate(out_names)}
            for c in range(NC)
        ]

    _STATE["runner"] = runner
    _STATE["internals"] = {
        "sharded": sharded, "in_names": in_names, "zero_shapes": zero_shapes,
        "mesh": mesh, "out_names": out_names, "out_avals": out_avals,
    }
    return runner


def kernel(vert, edge, W, a_l, a_r):
    in_maps = _prep_inputs(vert, edge, W, a_l, a_r)
    runner = _get_runner()
    results = runner(in_maps)
    return np.concatenate([results[c]["out"] for c in range(NC)], axis=0)
